# revision 8
# baseline (speedup 1.0000x reference)
"""CondensedLinearFineGrained on 8 TRN2 NeuronCores.

Math: out[b,o] = sum_k W[o,k] * input[b, mask[o,k]] + bias[o]
with B=256, IN_F=4096, OUT_F=4096, K=256.

Strategy
--------
Reformulate as a dense matmul:  out = input @ A^T  where
A[o,f] = sum_{k: mask[o,k]==f} W[o,k]  (duplicates within a row are summed);
bias is added on the host (B*OUT_F adds, negligible vs the 4.3G-MAC GEMM).

Sharding: output neurons, 512 per core. Per core the PE accumulates
psum[128b x 512o] over 32 f-tiles for each of 2 batch tiles (64 matmuls,
~14us at the warm 2.4GHz clock). A^T f-tiles are delivered two ways,
balanced so the DMA stream, the gpsimd scatter stream and the PE all
finish together:
  - N_DENSE tiles pre-densified on the host, bulk-DMA'd (128KB each).
  - The rest built on-device by gpsimd local_scatter from host-packed
    CSC (per-feature (o,weight) lists, deduped, -1-padded, int16), two
    tiles per instruction (~1.35us each).

Schedule notes (from perfetto analysis of the previous version):
  - The PE clock is HAM-throttled to 1.2GHz until it has been
    continuously busy for ~3.4us; back-to-back warmup matmuls from
    block start carry the activity until real tiles land, so the real
    stream runs almost entirely at 2.4GHz.
  - inT/atd/csc are packed in DRAM in PE-consumption order; scattered
    pairs are interleaved at the positions a static rate model predicts
    they are ready, so no stream runs ahead or stalls.
  - csc is dispatched first (gpsimd SWDGE shares the 16 DMA rings with
    the bulk traffic; dispatching it late queues it behind ~2MB).
  - batch-tile-1's matmuls lag batch-0 by LAG slots so psum0's
    copy-out + out-DMA overlap the last matmuls; copies are split
    across the vector and scalar engines; output is bf16.
"""

import numpy as np
import ml_dtypes

B = 256
IN_F = 4096
OUT_F = 4096
K = 256
N_CORES = 8
O_SH = OUT_F // N_CORES  # 512 output rows per core
NT = IN_F // 128         # 32 feature tiles
NB = B // 128            # 2 batch tiles

N_DENSE = 14             # host-densified tiles; NT - N_DENSE must be even
LAG = 3                  # batch-1 matmul lag (slots) for copy-out overlap
N_WARM = 10              # back-to-back dummy PE matmuls (HAM clock release)

_BF16 = ml_dtypes.bfloat16

_prog_cache = {}


# ---------------------------------------------------------------------------
# Static schedule: interleave dense tiles and scattered pairs in the order
# the PE consumes them, from a rate model of the three streams.
# ---------------------------------------------------------------------------
def _schedule(n_dense: int):
    nt_s = NT - n_dense
    npair = nt_s // 2
    assert nt_s % 2 == 0

    # rate model (us from block start)
    G0, GP = 1.6, 1.35          # gpsimd start latency, per-pair time
    MARGIN = 0.25

    def t_slot(k):              # PE consumption time of slot k
        if k <= 2:
            return 1.9 + 0.86 * k
        return 3.62 + 0.432 * (k - 2)

    ready = [G0 + GP * (j + 1) for j in range(npair)]

    slots = []                  # ('d', dense_idx) | ('s', pair_idx, half)
    di = 0
    j = 0
    while len(slots) < NT:
        k = len(slots)
        if j < npair and (t_slot(k) >= ready[j] + MARGIN or di >= n_dense):
            slots.append(('s', j, 0))
            slots.append(('s', j, 1))
            j += 1
        else:
            slots.append(('d', di))
            di += 1
    assert di == n_dense and j == npair and len(slots) == NT

    # chunk boundaries
    def cuts(total, sizes):
        out, p = [], 0
        for s in sizes:
            if p >= total:
                break
            out.append((p, min(p + s, total)))
            p += s
        if out and out[-1][1] < total:
            out.append((out[-1][1], total))
        return out

    in_chunks = cuts(NT, [2, 4, 6, 8, 8, 4])
    atd_chunks = cuts(n_dense, [2, 4, 4, 4])
    csc_chunks = cuts(npair, [2, npair - 2]) if npair > 2 else cuts(npair, [npair])
    return slots, in_chunks, atd_chunks, csc_chunks


def _build_program(wpad: int, n_dense: int):
    """Hand-scheduled SPMD program: explicit per-engine streams + semaphores."""
    key = (wpad, n_dense)
    if key in _prog_cache:
        return _prog_cache[key]

    from contextlib import ExitStack
    from concourse import bacc, mybir, library_config

    slots, in_chunks, atd_chunks, csc_chunks = _schedule(n_dense)
    nt_s = NT - n_dense
    npair = nt_s // 2

    def chunk_of(chunks, t):
        for c, (c0, c1) in enumerate(chunks):
            if c0 <= t < c1:
                return c
        raise AssertionError

    nc = bacc.Bacc("TRN2", target_bir_lowering=False, debug=False)
    dt = mybir.dt

    # inT is packed in consumption-slot order: inT[:, k, :] is the f-tile
    # consumed at slot k. atd is packed in dense-consumption order.
    inT_d = nc.dram_tensor("inT", [128, NT, B], dt.bfloat16, kind="ExternalInput")
    if npair:
        idx_d = nc.dram_tensor("cscidx", [128, npair, wpad], dt.int16,
                               kind="ExternalInput")
        val_d = nc.dram_tensor("cscval", [128, npair, wpad], dt.bfloat16,
                               kind="ExternalInput")
    if n_dense:
        atd_d = nc.dram_tensor("atd", [128, n_dense, O_SH], dt.bfloat16,
                               kind="ExternalInput")
    out_d = nc.dram_tensor("out", [NB, 128, O_SH], dt.bfloat16,
                           kind="ExternalOutput")

    inT_sb = nc.alloc_sbuf_tensor("inT_sb", [128, NT, B], dt.bfloat16).ap()
    warm_sb = nc.alloc_sbuf_tensor("warm_sb", [128, 384], dt.bfloat16).ap()
    if npair:
        idx_sb = nc.alloc_sbuf_tensor("idx_sb", [128, npair, wpad],
                                      dt.int16).ap()
        val_sb = nc.alloc_sbuf_tensor("val_sb", [128, npair, wpad],
                                      dt.bfloat16).ap()
        at_sb = nc.alloc_sbuf_tensor("at_sb", [128, npair, 2, O_SH],
                                     dt.bfloat16).ap()
    if n_dense:
        atd_sb = nc.alloc_sbuf_tensor("atd_sb", [128, n_dense, O_SH],
                                      dt.bfloat16).ap()
    outs_sb = [nc.alloc_sbuf_tensor(f"out_sb{i}", [128, O_SH], dt.bfloat16).ap()
               for i in range(NB)]

    psums = [nc.alloc_psum_tensor(f"ps{i}", [128, O_SH], dt.float32).ap()
             for i in range(NB)]
    ps_warm = nc.alloc_psum_tensor("ps_warm", [128, 256], dt.float32).ap()

    H = O_SH // 2  # copy half-width

    with ExitStack() as ctx:
        sem = lambda name: ctx.enter_context(nc.semaphore(name))
        # One semaphore per DMA: sub-transfers of back-to-back DMAs on one
        # queue can complete out of order, so prefix thresholds on a shared
        # semaphore would be unsound.
        s_in = [sem(f"s_in{c}") for c in range(len(in_chunks))]
        s_atd = [sem(f"s_atd{c}") for c in range(len(atd_chunks))]
        s_ci = [sem(f"s_ci{c}") for c in range(len(csc_chunks))] if npair else []
        s_cv = [sem(f"s_cv{c}") for c in range(len(csc_chunks))] if npair else []
        # out-DMA completion sems: incremented (the BIR verifier requires a
        # sem update on every DMA) but never waited in-kernel — the
        # runtime's queue drain covers output completion.
        s_od = sem("s_od")
        s_g = sem("s_g")    # scatter pairs published
        s_v = sem("s_v")    # warm consts ready
        s_ps = sem("s_ps")  # PE accumulation done per psum
        # one copy-done sem per psum: the two half-copies come from
        # different engines, so a shared counter could reach the waited
        # threshold with both increments from one psum's halves missing
        s_cp = [sem(f"s_cp{i}") for i in range(NB)]

        with nc.Block() as block:

            # ---- DMA dispatch: sync carries the inT stream (+ output at
            # the end), scalar carries the atd stream (+ ps1 half-copies at
            # the end). gpsimd dispatches csc first so it isn't queued
            # behind the bulk traffic on the shared rings.
            @block.sync
            def _(sy):
                for c, (c0, c1) in enumerate(in_chunks):
                    sy.dma_start(out=inT_sb[:, c0:c1, :],
                                 in_=inT_d[:, c0:c1, :]).then_inc(s_in[c], 16)
                for i in range(NB):
                    sy.wait_ge(s_cp[i], 2)
                    sy.dma_start(out=out_d[i],
                                 in_=outs_sb[i][:]).then_inc(s_od, 16)

            @block.scalar
            def _(sc):
                if n_dense:
                    for c, (c0, c1) in enumerate(atd_chunks):
                        sc.dma_start(out=atd_sb[:, c0:c1, :],
                                     in_=atd_d[:, c0:c1, :]).then_inc(s_atd[c], 16)


            @block.vector
            def _(v):
                v.memset(warm_sb[:], 0.125)
                v.drain()
                v.sem_inc(s_v, 1)
                for i in range(NB):
                    v.wait_ge(s_ps, i + 1)
                    v.tensor_copy(outs_sb[i][:, :H],
                                  psums[i][:, :H]).then_inc(s_cp[i], 1)
                    v.tensor_copy(outs_sb[i][:, H:],
                                  psums[i][:, H:]).then_inc(s_cp[i], 1)

            if npair:
                @block.gpsimd
                def _(g):
                    g.load_library(library_config.local_scatter)
                    for c, (c0, c1) in enumerate(csc_chunks):
                        g.dma_start(out=idx_sb[:, c0:c1, :],
                                    in_=idx_d[:, c0:c1, :]).then_inc(s_ci[c], 16)
                        g.dma_start(out=val_sb[:, c0:c1, :],
                                    in_=val_d[:, c0:c1, :]).then_inc(s_cv[c], 16)
                    for j in range(npair):
                        c = chunk_of(csc_chunks, j)
                        g.wait_ge(s_ci[c], 16)
                        g.wait_ge(s_cv[c], 16)
                        g.local_scatter(
                            at_sb[:, j],
                            val_sb[:, j],
                            idx_sb[:, j],
                            channels=128,
                            num_elems=2 * O_SH,
                            num_idxs=wpad,
                        ).then_inc(s_g, 1)

            @block.tensor
            def _(te):
                te.wait_ge(s_v, 1)
                # back-to-back dummy matmuls keep the PE busy from t~0 so
                # the HAM clock gate releases ~3.4us in; real matmuls then
                # run at 2.4GHz almost from the start
                for _ in range(N_WARM):
                    te.matmul(ps_warm[:], warm_sb[:, :128], warm_sb[:, 128:],
                              start=True, stop=True, skip_group_check=True)

                seen = set()
                g_thr = 0

                def wait_once(s):
                    if s.name not in seen:
                        te.wait_ge(s, 16)
                        seen.add(s.name)

                def rhs_of(k):
                    nonlocal g_thr
                    sl = slots[k]
                    if sl[0] == 'd':
                        wait_once(s_atd[chunk_of(atd_chunks, sl[1])])
                        return atd_sb[:, sl[1], :]
                    j = sl[1]
                    if j + 1 > g_thr:
                        te.wait_ge(s_g, j + 1)
                        g_thr = j + 1
                    return at_sb[:, j, sl[2], :]

                def mm(k, i):
                    last = k == NT - 1
                    m = te.matmul(psums[i][:],
                                  inT_sb[:, k, 128 * i:128 * (i + 1)],
                                  rhs_of(k),
                                  start=(k == 0), stop=last)
                    if last:
                        m.then_inc(s_ps, 1)

                # batch-0 leads, batch-1 lags LAG slots so psum0's
                # copy-out overlaps the last matmuls
                for k in range(NT):
                    wait_once(s_in[chunk_of(in_chunks, k)])
                    mm(k, 0)
                    if k >= LAG:
                        mm(k - LAG, 1)
                for k in range(NT - LAG, NT):
                    mm(k, 1)

        # after the work block's all-engine barrier, recycle semaphores so
        # the next execution of this NEFF starts from zero
        # s_od deliberately NOT recycled: the out DMAs are still in flight
        # when the recycle block runs (nothing in-kernel waits on them), so
        # clearing would race the DMA's increment. Nothing depends on its
        # value either.
        all_sems = s_in + s_atd + s_ci + s_cv + s_cp + [s_g, s_v, s_ps]
        with nc.Block() as block2:

            @block2.sync
            def _(sy):
                for s in all_sems:
                    sy.sem_clear(s)

    nc.compile()
    _prog_cache[key] = nc
    return nc


def _prepare(input, condensed_weight, input_mask, bias):
    """Host-side repack: dedupe + CSC-bin the sparse weights, cast/transpose
    the activations, pack everything in PE-consumption order."""
    slots, _, _, _ = _schedule(N_DENSE)
    nt_s = NT - N_DENSE
    npair = nt_s // 2

    # dedupe (o, f) pairs, summing weights in f64
    o_idx = np.repeat(np.arange(OUT_F, dtype=np.int64), K)
    f_idx = input_mask.ravel().astype(np.int64)
    w = condensed_weight.ravel()
    key = (o_idx << 12) | f_idx
    uk, inv = np.unique(key, return_inverse=True)
    sums = np.bincount(inv, weights=w.astype(np.float64))
    o_u = (uk >> 12).astype(np.int64)
    f_u = (uk & (IN_F - 1)).astype(np.int64)
    v_u = sums.astype(np.float32)

    core = o_u // O_SH
    o_loc = o_u % O_SH
    t_id = f_u // 128
    p_f = f_u % 128

    # --- choose which global f-tiles are scattered: the nt_s tiles with the
    # smallest max-per-(core,partition) count give the smallest wpad; pair
    # big-with-small to balance pair sums.
    cnt = np.zeros((N_CORES * 128, NT), dtype=np.int64)
    np.add.at(cnt, (core * 128 + p_f, t_id), 1)
    tile_score = cnt.max(axis=0)
    order_by_score = np.argsort(tile_score, kind="stable")
    scat_tiles = np.sort(order_by_score[:nt_s])
    dense_tiles = np.sort(order_by_score[nt_s:])
    assert len(dense_tiles) == N_DENSE

    # pairing: among scattered tiles, pair lowest-score with highest-score
    ss = sorted(scat_tiles, key=lambda t: tile_score[t])
    pairs = [(ss[i], ss[nt_s - 1 - i]) for i in range(npair)]

    # maps: global tile -> (kind, index, half)
    tile_map = {}
    for j, (ta, tb) in enumerate(pairs):
        tile_map[ta] = ('s', j, 0)
        tile_map[tb] = ('s', j, 1)
    for d, t in enumerate(dense_tiles):
        tile_map[t] = ('d', d)

    # consumption slot of each (kind, idx[, half])
    slot_of = {}
    for k, sl in enumerate(slots):
        slot_of[sl] = k
    # global tile -> consumption slot
    tile_slot = np.empty(NT, dtype=np.int64)
    for t in range(NT):
        m = tile_map[t]
        tile_slot[t] = slot_of[m if m[0] == 's' else ('d', m[1])]

    # inT packed by consumption slot: inT[p, k, b] = input[b, 128*g(k) + p]
    # where g(k) is the global tile consumed at slot k.
    slot_tile = np.empty(NT, dtype=np.int64)
    slot_tile[tile_slot] = np.arange(NT)
    inT = np.ascontiguousarray(
        input.astype(_BF16).T.reshape(NT, 128, B)[slot_tile].transpose(1, 0, 2))

    # dense A^T tiles, packed in dense-consumption order
    d_of_tile = np.full(NT, -1, dtype=np.int64)
    for t in range(NT):
        if tile_map[t][0] == 'd':
            d_of_tile[t] = tile_map[t][1]
    dense_m = d_of_tile[t_id] >= 0
    if N_DENSE:
        atd = np.zeros((N_CORES, 128, N_DENSE, O_SH), dtype=_BF16)
        atd[core[dense_m], p_f[dense_m], d_of_tile[t_id[dense_m]],
            o_loc[dense_m]] = v_u[dense_m]

    wpad = 2
    if npair:
        pair_of_tile = np.full(NT, -1, dtype=np.int64)
        half_of_tile = np.zeros(NT, dtype=np.int64)
        for t in range(NT):
            if tile_map[t][0] == 's':
                pair_of_tile[t] = tile_map[t][1]
                half_of_tile[t] = tile_map[t][2]
        sm = ~dense_m
        s_core, s_p, s_o, s_v = core[sm], p_f[sm], o_loc[sm], v_u[sm]
        s_pair = pair_of_tile[t_id[sm]]
        s_idx = s_o + O_SH * half_of_tile[t_id[sm]]
        # rank of each entry within its (core, partition, pair) group
        g = (s_core * 128 + s_p) * npair + s_pair
        order = np.argsort(g, kind="stable")
        gs = g[order]
        change = np.r_[True, gs[1:] != gs[:-1]]
        seg_start = np.flatnonzero(change)
        seg_id = np.cumsum(change) - 1
        rank = np.arange(gs.size) - seg_start[seg_id]

        maxc = int(rank.max()) + 1 if gs.size else 0
        wpad = max(2, (maxc + 1) // 2 * 2)

        idx_arr = np.full((N_CORES, 128, npair, wpad), -1, dtype=np.int16)
        val_arr = np.zeros((N_CORES, 128, npair, wpad), dtype=_BF16)
        idx_arr[s_core[order], s_p[order], s_pair[order], rank] = \
            s_idx[order].astype(np.int16)
        val_arr[s_core[order], s_p[order], s_pair[order], rank] = s_v[order]

    in_maps = []
    for c in range(N_CORES):
        m = {"inT": inT}
        if npair:
            m["cscidx"] = np.ascontiguousarray(idx_arr[c])
            m["cscval"] = np.ascontiguousarray(val_arr[c])
        if N_DENSE:
            m["atd"] = np.ascontiguousarray(atd[c])
        in_maps.append(m)
    return in_maps, wpad


def kernel(input, condensed_weight, input_mask, bias,
           _run_kwargs=None, _res_box=None):
    """Full inputs in, full output out. Shards over 8 NeuronCores inside."""
    from concourse.bass_utils import run_bass_kernel_spmd

    input = np.asarray(input)
    bias = np.asarray(bias)
    in_maps, wpad = _prepare(
        input, np.asarray(condensed_weight),
        np.asarray(input_mask), bias)
    nc = _build_program(wpad, N_DENSE)

    res = run_bass_kernel_spmd(nc, in_maps, list(range(N_CORES)),
                               **(_run_kwargs or {}))
    if _res_box is not None:
        _res_box["results"] = res

    out = np.concatenate(
        [np.asarray(res.results[c]["out"]).reshape(B, O_SH).astype(np.float32)
         for c in range(N_CORES)], axis=1)
    return out + bias[None, :].astype(np.float32)


# revision 14
# speedup vs baseline: 1.0857x; 1.0857x over previous
"""CondensedLinearFineGrained on 8 TRN2 NeuronCores.

Math: out[b,o] = sum_k W[o,k] * input[b, mask[o,k]] + bias[o]
with B=256, IN_F=4096, OUT_F=4096, K=256.

Strategy
--------
Reformulate as a dense matmul:  out = input @ A^T  where
A[o,f] = sum_{k: mask[o,k]==f} W[o,k]  (duplicates within a row are summed);
bias is added on the host (B*OUT_F adds, negligible vs the 4.3G-MAC GEMM).

Sharding: output neurons, 512 per core. Per core the PE accumulates
psum[128b x 512o] over 32 f-tiles for each of 2 batch tiles (64 matmuls,
~14us at the warm 2.4GHz clock). A^T f-tiles are delivered two ways,
balanced so the DMA stream, the gpsimd scatter stream and the PE all
finish together:
  - N_DENSE tiles pre-densified on the host, bulk-DMA'd (128KB each).
  - The rest built on-device by gpsimd local_scatter from host-packed
    CSC (per-feature (o,weight) lists, deduped, -1-padded, int16), two
    tiles per instruction (~1.35us each).

Schedule notes (from perfetto analysis of the previous version):
  - The PE clock is HAM-throttled to 1.2GHz until it has been
    continuously busy for ~3.4us; back-to-back warmup matmuls from
    block start carry the activity until real tiles land, so the real
    stream runs almost entirely at 2.4GHz.
  - inT/atd/csc are packed in DRAM in PE-consumption order; scattered
    pairs are interleaved at the positions a static rate model predicts
    they are ready, so no stream runs ahead or stalls.
  - csc is dispatched first (gpsimd SWDGE shares the 16 DMA rings with
    the bulk traffic; dispatching it late queues it behind ~2MB).
  - batch-tile-1's matmuls lag batch-0 by LAG slots so psum0's
    copy-out + out-DMA overlap the last matmuls; copies are split
    across the vector and scalar engines; output is bf16.
"""

import numpy as np
import ml_dtypes

B = 256
IN_F = 4096
OUT_F = 4096
K = 256
N_CORES = 8
O_SH = OUT_F // N_CORES  # 512 output rows per core
NT = IN_F // 128         # 32 feature tiles
NB = B // 128            # 2 batch tiles

N_DENSE = 14             # host-densified tiles; NT - N_DENSE must be even
LAG = 3                  # batch-1 matmul lag (slots) for copy-out overlap
N_WARM = 12              # back-to-back dummy PE matmuls (HAM clock release)

_BF16 = ml_dtypes.bfloat16

_prog_cache = {}


# ---------------------------------------------------------------------------
# Static schedule: interleave dense tiles and scattered pairs in the order
# the PE consumes them, from a rate model of the three streams.
# ---------------------------------------------------------------------------
def _schedule(n_dense: int):
    nt_s = NT - n_dense
    npair = nt_s // 2
    assert nt_s % 2 == 0

    # rate model (us from block start), calibrated from perfetto: DMA
    # ring-start ~1.5us, first tile available ~3.0, HAM un-throttle ~4.0,
    # warm slot rate 0.445us (2 matmuls), scatter start ~2.2 + 1.39/pair
    G0, GP = 2.2, 1.39
    MARGIN = 0.25

    def t_slot(k):              # PE consumption time of slot k
        if k <= 2:
            return 3.0 + 0.89 * k
        return 4.78 + 0.445 * (k - 2)

    ready = [G0 + GP * (j + 1) for j in range(npair)]

    slots = []                  # ('d', dense_idx) | ('s', pair_idx, half)
    di = 0
    j = 0
    while len(slots) < NT:
        k = len(slots)
        if j < npair and (t_slot(k) >= ready[j] + MARGIN or di >= n_dense):
            slots.append(('s', j, 0))
            slots.append(('s', j, 1))
            j += 1
        else:
            slots.append(('d', di))
            di += 1
    assert di == n_dense and j == npair and len(slots) == NT

    # chunk boundaries
    def cuts(total, sizes):
        out, p = [], 0
        for s in sizes:
            if p >= total:
                break
            out.append((p, min(p + s, total)))
            p += s
        if out and out[-1][1] < total:
            out.append((out[-1][1], total))
        return out

    in_chunks = cuts(NT, [1, 3, 4, 8, 8, 8])
    atd_chunks = cuts(n_dense, [1, 2, 3, 4, 4])
    csc_chunks = cuts(npair, [3, npair - 3]) if npair > 3 else cuts(npair, [npair])
    return slots, in_chunks, atd_chunks, csc_chunks


def _build_program(wpad: int, n_dense: int):
    """Hand-scheduled SPMD program: explicit per-engine streams + semaphores."""
    key = (wpad, n_dense)
    if key in _prog_cache:
        return _prog_cache[key]

    from contextlib import ExitStack
    from concourse import bacc, mybir, library_config

    slots, in_chunks, atd_chunks, csc_chunks = _schedule(n_dense)
    nt_s = NT - n_dense
    npair = nt_s // 2

    def chunk_of(chunks, t):
        for c, (c0, c1) in enumerate(chunks):
            if c0 <= t < c1:
                return c
        raise AssertionError

    nc = bacc.Bacc("TRN2", target_bir_lowering=False, debug=False)
    dt = mybir.dt

    # inT is packed in consumption-slot order: inT[:, k, :] is the f-tile
    # consumed at slot k. atd is packed in dense-consumption order.
    inT_d = nc.dram_tensor("inT", [128, NT, B], dt.bfloat16, kind="ExternalInput")
    if npair:
        idx_d = nc.dram_tensor("cscidx", [128, npair, wpad], dt.int16,
                               kind="ExternalInput")
        val_d = nc.dram_tensor("cscval", [128, npair, wpad], dt.bfloat16,
                               kind="ExternalInput")
    if n_dense:
        atd_d = nc.dram_tensor("atd", [128, n_dense, O_SH], dt.bfloat16,
                               kind="ExternalInput")
    out_d = nc.dram_tensor("out", [NB, 128, O_SH], dt.bfloat16,
                           kind="ExternalOutput")

    inT_sb = nc.alloc_sbuf_tensor("inT_sb", [128, NT, B], dt.bfloat16).ap()
    warm_sb = nc.alloc_sbuf_tensor("warm_sb", [128, 384], dt.bfloat16).ap()
    if npair:
        idx_sb = nc.alloc_sbuf_tensor("idx_sb", [128, npair, wpad],
                                      dt.int16).ap()
        val_sb = nc.alloc_sbuf_tensor("val_sb", [128, npair, wpad],
                                      dt.bfloat16).ap()
        at_sb = nc.alloc_sbuf_tensor("at_sb", [128, npair, 2, O_SH],
                                     dt.bfloat16).ap()
    if n_dense:
        atd_sb = nc.alloc_sbuf_tensor("atd_sb", [128, n_dense, O_SH],
                                      dt.bfloat16).ap()
    outs_sb = [nc.alloc_sbuf_tensor(f"out_sb{i}", [128, O_SH], dt.bfloat16).ap()
               for i in range(NB)]

    psums = [nc.alloc_psum_tensor(f"ps{i}", [128, O_SH], dt.float32).ap()
             for i in range(NB)]
    ps_warm = nc.alloc_psum_tensor("ps_warm", [128, 256], dt.float32).ap()

    H = O_SH // 2  # copy half-width

    with ExitStack() as ctx:
        sem = lambda name: ctx.enter_context(nc.semaphore(name))
        # One semaphore per DMA: sub-transfers of back-to-back DMAs on one
        # queue can complete out of order, so prefix thresholds on a shared
        # semaphore would be unsound.
        s_in = [sem(f"s_in{c}") for c in range(len(in_chunks))]
        s_atd = [sem(f"s_atd{c}") for c in range(len(atd_chunks))]
        s_ci = [sem(f"s_ci{c}") for c in range(len(csc_chunks))] if npair else []
        s_cv = [sem(f"s_cv{c}") for c in range(len(csc_chunks))] if npair else []
        # out-DMA completion sems: incremented (the BIR verifier requires a
        # sem update on every DMA) but never waited in-kernel — the
        # runtime's queue drain covers output completion.
        s_od = sem("s_od")
        s_g = sem("s_g")    # scatter pairs published
        s_v = sem("s_v")    # warm consts ready
        s_ps = sem("s_ps")  # PE accumulation done per psum
        # one copy-done sem per psum: the two half-copies come from
        # different engines, so a shared counter could reach the waited
        # threshold with both increments from one psum's halves missing
        s_cp = [sem(f"s_cp{i}") for i in range(NB)]

        with nc.Block() as block:

            # ---- DMA dispatch: the 16 rings execute in global enqueue
            # order, so csc chunk 0 (the scatter-chain critical path) is
            # dispatched FIRST on sync, before any bulk traffic. sync then
            # carries the inT stream (+ output at the end); scalar carries
            # the atd stream + csc chunk 1. gpsimd dispatches nothing (its
            # ucode-library load would delay the csc DMAs by ~2.5us).
            @block.sync
            def _(sy):
                if npair:
                    c0, c1 = csc_chunks[0]
                    sy.dma_start(out=idx_sb[:, c0:c1, :],
                                 in_=idx_d[:, c0:c1, :]).then_inc(s_ci[0], 16)
                    sy.dma_start(out=val_sb[:, c0:c1, :],
                                 in_=val_d[:, c0:c1, :]).then_inc(s_cv[0], 16)
                for c, (c0, c1) in enumerate(in_chunks):
                    sy.dma_start(out=inT_sb[:, c0:c1, :],
                                 in_=inT_d[:, c0:c1, :]).then_inc(s_in[c], 16)
                for i in range(NB):
                    sy.wait_ge(s_cp[i], 2)
                    sy.dma_start(out=out_d[i],
                                 in_=outs_sb[i][:]).then_inc(s_od, 16)

            @block.scalar
            def _(sc):
                if n_dense:
                    for c, (c0, c1) in enumerate(atd_chunks):
                        sc.dma_start(out=atd_sb[:, c0:c1, :],
                                     in_=atd_d[:, c0:c1, :]).then_inc(s_atd[c], 16)
                        if c == 0 and npair and len(csc_chunks) > 1:
                            d0, d1 = csc_chunks[1]
                            sc.dma_start(out=idx_sb[:, d0:d1, :],
                                         in_=idx_d[:, d0:d1, :]).then_inc(
                                             s_ci[1], 16)
                            sc.dma_start(out=val_sb[:, d0:d1, :],
                                         in_=val_d[:, d0:d1, :]).then_inc(
                                             s_cv[1], 16)


            @block.vector
            def _(v):
                v.memset(warm_sb[:], 0.125)
                v.drain()
                v.sem_inc(s_v, 1)
                for i in range(NB):
                    v.wait_ge(s_ps, i + 1)
                    v.tensor_copy(outs_sb[i][:, :H],
                                  psums[i][:, :H]).then_inc(s_cp[i], 1)
                    v.tensor_copy(outs_sb[i][:, H:],
                                  psums[i][:, H:]).then_inc(s_cp[i], 1)

            if npair:
                @block.gpsimd
                def _(g):
                    g.load_library(library_config.local_scatter)
                    seen_g = set()
                    for j in range(npair):
                        c = chunk_of(csc_chunks, j)
                        if c not in seen_g:
                            g.wait_ge(s_ci[c], 16)
                            g.wait_ge(s_cv[c], 16)
                            seen_g.add(c)
                        g.local_scatter(
                            at_sb[:, j],
                            val_sb[:, j],
                            idx_sb[:, j],
                            channels=128,
                            num_elems=2 * O_SH,
                            num_idxs=wpad,
                        ).then_inc(s_g, 1)

            @block.tensor
            def _(te):
                te.wait_ge(s_v, 1)
                # back-to-back dummy matmuls keep the PE busy from t~0 so
                # the HAM clock gate releases ~3.4us in; real matmuls then
                # run at 2.4GHz almost from the start
                for _ in range(N_WARM):
                    te.matmul(ps_warm[:], warm_sb[:, :128], warm_sb[:, 128:],
                              start=True, stop=True, skip_group_check=True)

                seen = set()
                g_thr = 0

                def wait_once(s):
                    if s.name not in seen:
                        te.wait_ge(s, 16)
                        seen.add(s.name)

                def rhs_of(k):
                    nonlocal g_thr
                    sl = slots[k]
                    if sl[0] == 'd':
                        wait_once(s_atd[chunk_of(atd_chunks, sl[1])])
                        return atd_sb[:, sl[1], :]
                    j = sl[1]
                    if j + 1 > g_thr:
                        te.wait_ge(s_g, j + 1)
                        g_thr = j + 1
                    return at_sb[:, j, sl[2], :]

                def mm(k, i):
                    last = k == NT - 1
                    m = te.matmul(psums[i][:],
                                  inT_sb[:, k, 128 * i:128 * (i + 1)],
                                  rhs_of(k),
                                  start=(k == 0), stop=last)
                    if last:
                        m.then_inc(s_ps, 1)

                # batch-0 leads, batch-1 lags LAG slots so psum0's
                # copy-out overlaps the last matmuls
                for k in range(NT):
                    wait_once(s_in[chunk_of(in_chunks, k)])
                    mm(k, 0)
                    if k >= LAG:
                        mm(k - LAG, 1)
                for k in range(NT - LAG, NT):
                    mm(k, 1)

        # after the work block's all-engine barrier, recycle semaphores so
        # the next execution of this NEFF starts from zero
        # s_od deliberately NOT recycled: the out DMAs are still in flight
        # when the recycle block runs (nothing in-kernel waits on them), so
        # clearing would race the DMA's increment. Nothing depends on its
        # value either. Clears are split across engines to shorten the
        # recycle block.
        all_sems = s_in + s_atd + s_ci + s_cv + s_cp + [s_g, s_v, s_ps]
        with nc.Block() as block2:

            @block2.sync
            def _(sy):
                for s in all_sems[0::3]:
                    sy.sem_clear(s)

            @block2.scalar
            def _(sc):
                for s in all_sems[1::3]:
                    sc.sem_clear(s)

            @block2.vector
            def _(v):
                for s in all_sems[2::3]:
                    v.sem_clear(s)

    nc.compile()
    _prog_cache[key] = nc
    return nc


def _prepare(input, condensed_weight, input_mask, bias):
    """Host-side repack: dedupe + CSC-bin the sparse weights, cast/transpose
    the activations, pack everything in PE-consumption order."""
    slots, _, _, _ = _schedule(N_DENSE)
    nt_s = NT - N_DENSE
    npair = nt_s // 2

    # dedupe (o, f) pairs, summing weights in f64
    o_idx = np.repeat(np.arange(OUT_F, dtype=np.int64), K)
    f_idx = input_mask.ravel().astype(np.int64)
    w = condensed_weight.ravel()
    key = (o_idx << 12) | f_idx
    uk, inv = np.unique(key, return_inverse=True)
    sums = np.bincount(inv, weights=w.astype(np.float64))
    o_u = (uk >> 12).astype(np.int64)
    f_u = (uk & (IN_F - 1)).astype(np.int64)
    v_u = sums.astype(np.float32)

    core = o_u // O_SH
    o_loc = o_u % O_SH
    t_id = f_u // 128
    p_f = f_u % 128

    # --- choose which global f-tiles are scattered: the nt_s tiles with the
    # smallest max-per-(core,partition) count give the smallest wpad; pair
    # big-with-small to balance pair sums.
    cnt = np.zeros((N_CORES * 128, NT), dtype=np.int64)
    np.add.at(cnt, (core * 128 + p_f, t_id), 1)
    tile_score = cnt.max(axis=0)
    order_by_score = np.argsort(tile_score, kind="stable")
    scat_tiles = np.sort(order_by_score[:nt_s])
    dense_tiles = np.sort(order_by_score[nt_s:])
    assert len(dense_tiles) == N_DENSE

    # pairing: among scattered tiles, pair lowest-score with highest-score
    ss = sorted(scat_tiles, key=lambda t: tile_score[t])
    pairs = [(ss[i], ss[nt_s - 1 - i]) for i in range(npair)]

    # maps: global tile -> (kind, index, half)
    tile_map = {}
    for j, (ta, tb) in enumerate(pairs):
        tile_map[ta] = ('s', j, 0)
        tile_map[tb] = ('s', j, 1)
    for d, t in enumerate(dense_tiles):
        tile_map[t] = ('d', d)

    # consumption slot of each (kind, idx[, half])
    slot_of = {}
    for k, sl in enumerate(slots):
        slot_of[sl] = k
    # global tile -> consumption slot
    tile_slot = np.empty(NT, dtype=np.int64)
    for t in range(NT):
        m = tile_map[t]
        tile_slot[t] = slot_of[m if m[0] == 's' else ('d', m[1])]

    # inT packed by consumption slot: inT[p, k, b] = input[b, 128*g(k) + p]
    # where g(k) is the global tile consumed at slot k.
    slot_tile = np.empty(NT, dtype=np.int64)
    slot_tile[tile_slot] = np.arange(NT)
    inT = np.ascontiguousarray(
        input.astype(_BF16).T.reshape(NT, 128, B)[slot_tile].transpose(1, 0, 2))

    # dense A^T tiles, packed in dense-consumption order
    d_of_tile = np.full(NT, -1, dtype=np.int64)
    for t in range(NT):
        if tile_map[t][0] == 'd':
            d_of_tile[t] = tile_map[t][1]
    dense_m = d_of_tile[t_id] >= 0
    if N_DENSE:
        atd = np.zeros((N_CORES, 128, N_DENSE, O_SH), dtype=_BF16)
        atd[core[dense_m], p_f[dense_m], d_of_tile[t_id[dense_m]],
            o_loc[dense_m]] = v_u[dense_m]

    wpad = 2
    if npair:
        pair_of_tile = np.full(NT, -1, dtype=np.int64)
        half_of_tile = np.zeros(NT, dtype=np.int64)
        for t in range(NT):
            if tile_map[t][0] == 's':
                pair_of_tile[t] = tile_map[t][1]
                half_of_tile[t] = tile_map[t][2]
        sm = ~dense_m
        s_core, s_p, s_o, s_v = core[sm], p_f[sm], o_loc[sm], v_u[sm]
        s_pair = pair_of_tile[t_id[sm]]
        s_idx = s_o + O_SH * half_of_tile[t_id[sm]]
        # rank of each entry within its (core, partition, pair) group
        g = (s_core * 128 + s_p) * npair + s_pair
        order = np.argsort(g, kind="stable")
        gs = g[order]
        change = np.r_[True, gs[1:] != gs[:-1]]
        seg_start = np.flatnonzero(change)
        seg_id = np.cumsum(change) - 1
        rank = np.arange(gs.size) - seg_start[seg_id]

        maxc = int(rank.max()) + 1 if gs.size else 0
        wpad = max(2, (maxc + 1) // 2 * 2)

        idx_arr = np.full((N_CORES, 128, npair, wpad), -1, dtype=np.int16)
        val_arr = np.zeros((N_CORES, 128, npair, wpad), dtype=_BF16)
        idx_arr[s_core[order], s_p[order], s_pair[order], rank] = \
            s_idx[order].astype(np.int16)
        val_arr[s_core[order], s_p[order], s_pair[order], rank] = s_v[order]

    in_maps = []
    for c in range(N_CORES):
        m = {"inT": inT}
        if npair:
            m["cscidx"] = np.ascontiguousarray(idx_arr[c])
            m["cscval"] = np.ascontiguousarray(val_arr[c])
        if N_DENSE:
            m["atd"] = np.ascontiguousarray(atd[c])
        in_maps.append(m)
    return in_maps, wpad


def kernel(input, condensed_weight, input_mask, bias,
           _run_kwargs=None, _res_box=None):
    """Full inputs in, full output out. Shards over 8 NeuronCores inside."""
    from concourse.bass_utils import run_bass_kernel_spmd

    input = np.asarray(input)
    bias = np.asarray(bias)
    in_maps, wpad = _prepare(
        input, np.asarray(condensed_weight),
        np.asarray(input_mask), bias)
    nc = _build_program(wpad, N_DENSE)

    res = run_bass_kernel_spmd(nc, in_maps, list(range(N_CORES)),
                               **(_run_kwargs or {}))
    if _res_box is not None:
        _res_box["results"] = res

    out = np.concatenate(
        [np.asarray(res.results[c]["out"]).reshape(B, O_SH).astype(np.float32)
         for c in range(N_CORES)], axis=1)
    return out + bias[None, :].astype(np.float32)
